# revision 18
# baseline (speedup 1.0000x reference)
"""Trainium2 Bass kernel for nn_KeplerDiffEq.

Computes, per orbit (4 orbits on 4 SBUF partitions):
  E = Kepler solve (Newton, seed E0 = M + e*sinM, 2 iterations, final
  trig via first-order rotation sin(E1-d) ~ sinE1 - d*cosE1)
  dr/ddr via the orbital-plane -> inertial rotation, out = [dr | ddr]  [4,6]

Accuracy (vs the 2000-step damped-Newton f32 reference, worst case over
M in [0,1)): Newton-2+rotate ~6e-4, magic-rsqrt+1NR ~3.5e-3 -> ~4e-3
total, vs the 2e-2 gate.

Schedule: the serial Newton chain (3 Sin ACTs + ~21 tiny vector ops)
runs on Scalar+Vector; everything independent of E (rotation matrix C,
V = [3a, 3a(1-e^2), x^2+y^2], magic-rsqrt refine, u2 = -mm^2 a^3
rsqrt(xx+yy)*[x,y]) runs concurrently on GpSimd (tensor_tensor only --
Pool rejects TensorScalarPtr and 32-bit shifts, so constants arrive as
input lanes and the 2-op magic seed runs in a Vector ACT-wait gap).
The mm^2 a^3 product chain runs on the otherwise-idle Scalar engine via
Square/Copy activations. Host packs angles with -pi/2 offsets pre-added
so cos comes from the single Sin table (no Sqrt table).

Sharding: problem is tiny ("too small to shard") -> replicated SPMD on
all 8 cores; core 0's output is returned.
"""
import sys

if "/opt/trn_rl_repo" not in sys.path:
    sys.path.insert(0, "/opt/trn_rl_repo")

import numpy as np

N_ORBITS = 4
N_IN = 25
N_OUT = 6
HALF_PI = float(np.float32(np.pi / 2))
MAGIC = 0x5F3759DF

_cache = {}


def _build():
    import concourse.tile as tile
    from concourse import bacc, mybir

    AF = mybir.ActivationFunctionType
    ALU = mybir.AluOpType
    F32 = mybir.dt.float32
    I32 = mybir.dt.int32
    P = N_ORBITS

    nc = bacc.Bacc("TRN2", target_bir_lowering=False, debug=False)
    IN = nc.dram_tensor("inp", [P, N_IN], F32, kind="ExternalInput")
    OUT = nc.dram_tensor("out", [P, N_OUT], F32, kind="ExternalOutput")

    with tile.TileContext(nc) as tc:
        with tc.tile_pool(name="p", bufs=1) as pool:
            tin = pool.tile([P, N_IN], F32, tag="tin")
            nc.sync.dma_start(tin[:], IN.ap())

            m_ap = tin[:, 0:1]
            e_ap = tin[:, 11:12]
            a_ap = tin[:, 12:13]
            mm_ap = tin[:, 13:14]
            xy_ap = tin[:, 14:16]
            offs2 = tin[:, 16:18]    # [0, -pi/2]
            sgn_pm = tin[:, 18:20]   # [-1, +1]
            neg1 = tin[:, 18:19]
            c3 = tin[:, 20:21]       # 3.0
            nhalf = tin[:, 21:22]    # -0.5
            c1p5 = tin[:, 22:23]     # 1.5
            nxy = tin[:, 23:25]      # [-x, -y]

            # T = sin(angles): cols 0..10 =
            # [M, w-pi/2, w, W, W-pi/2, w, pi/2-w, i, i-pi/2, W-pi/2, W]
            # -> [sM, n_w, s_w, s_W, n_W, s_w, c_w, s_i, n_i, n_W, s_W]
            # (s_* = sin, n_* = -cos, c_* = +cos)
            T = pool.tile([P, 11], F32, tag="T")
            nc.scalar.activation(T[:], tin[:, 0:11], AF.Sin)

            # ---- GpSimd side-channel (everything independent of E) ----
            # V = [3a, 3a(1-e^2), x^2+y^2, F0]; lane 3 is the Newton seed
            # F0 = e*sinM -- writing it here makes the magic-rsqrt shift
            # (which reads V[:, 0:4]) depend on F0, pinning the scheduler
            # so the critical-chain seed runs before the rsqrt side work.
            V = pool.tile([P, 4], F32, tag="V")
            nc.gpsimd.tensor_tensor(out=V[:, 0:1], in0=a_ap, in1=c3,
                                    op=ALU.mult)
            e2g = pool.tile([P, 1], F32, tag="e2g")
            nc.gpsimd.tensor_tensor(out=e2g[:], in0=e_ap, in1=e_ap,
                                    op=ALU.mult)
            tv = pool.tile([P, 1], F32, tag="tv")   # e^2 * 3a
            nc.gpsimd.tensor_tensor(out=tv[:], in0=e2g[:], in1=V[:, 0:1],
                                    op=ALU.mult)
            nc.gpsimd.tensor_tensor(out=V[:, 1:2], in0=V[:, 0:1], in1=tv[:],
                                    op=ALU.subtract)
            ne = pool.tile([P, 1], F32, tag="ne")   # -e
            nc.gpsimd.tensor_tensor(out=ne[:], in0=e_ap, in1=neg1,
                                    op=ALU.mult)
            nae = pool.tile([P, 1], F32, tag="nae")  # -a*e
            nc.gpsimd.tensor_tensor(out=nae[:], in0=ne[:], in1=a_ap,
                                    op=ALU.mult)
            # rotation-matrix pieces that only need T
            C = pool.tile([P, 6], F32, tag="C")  # [c11,c21,c31,c12,c22,c32]
            Cv = C[:].rearrange("p (h j) -> p h j", h=2)[:, :, 0:2]
            nc.gpsimd.tensor_tensor(out=C[:, 2:3], in0=T[:, 2:3],
                                    in1=T[:, 7:8], op=ALU.mult)  # c31=sw*si
            nw2 = pool.tile([P, 2], F32, tag="nw2")  # [cw, -sw]
            nc.gpsimd.tensor_tensor(out=nw2[:], in0=T[:, 1:3],
                                    in1=neg1.broadcast_to([P, 2]),
                                    op=ALU.mult)
            LWc = pool.tile([P, 2], F32, tag="LWc")  # [cW, sW]
            nc.gpsimd.tensor_tensor(out=LWc[:], in0=T[:, 9:11], in1=sgn_pm,
                                    op=ALU.mult)
            nc.gpsimd.tensor_tensor(out=C[:, 5:6], in0=nw2[:, 0:1],
                                    in1=T[:, 7:8], op=ALU.mult)  # c32=cw*si
            # (magic-rsqrt refine emitted after the Vector seed ops below --
            # tile deps are tracked in emission order, so a pool read of Y
            # emitted before Y's write would silently get no dependency)
            Y = pool.tile([P, 4], F32, tag="Y")

            # ---- Vector: idle-start fillers, then the Newton chain ----
            sq2 = pool.tile([P, 2], F32, tag="sq2")
            nc.vector.tensor_tensor(out=sq2[:], in0=xy_ap, in1=xy_ap,
                                    op=ALU.mult)
            nc.vector.tensor_tensor(out=V[:, 2:3], in0=sq2[:, 0:1],
                                    in1=sq2[:, 1:2], op=ALU.add)

            # seed: F0 = e*sinM -> V[:, 3]; EE0 = [F0, F0 - pi/2]
            F0 = V[:, 3:4]
            nc.vector.tensor_scalar(out=F0, in0=T[:, 0:1], scalar1=e_ap,
                                    scalar2=None, op0=ALU.mult)
            EE0 = pool.tile([P, 2], F32, tag="EE0")
            nc.vector.tensor_tensor(out=EE0[:], in0=F0.broadcast_to([P, 2]),
                                    in1=offs2, op=ALU.add)
            S1 = pool.tile([P, 2], F32, tag="S1")
            nc.scalar.activation(S1[:], EE0[:], AF.Sin, bias=m_ap)

            # gap fillers during S1: magic-rsqrt seed (shift needs Vector)
            sh = pool.tile([P, 4], I32, tag="sh")
            nc.vector.tensor_scalar(out=sh[:], in0=V[:].bitcast(I32),
                                    scalar1=1, scalar2=None,
                                    op0=ALU.logical_shift_right)
            nc.vector.tensor_scalar(out=Y[:].bitcast(I32), in0=sh[:],
                                    scalar1=MAGIC, scalar2=-1,
                                    op0=ALU.subtract, op1=ALU.mult)

            # GpSimd: magic-rsqrt Newton-Raphson refine + SQpm
            nr = pool.tile([P, 4], F32, tag="nr")
            nc.gpsimd.tensor_tensor(out=nr[:], in0=Y[:], in1=Y[:],
                                    op=ALU.mult)
            nc.gpsimd.tensor_tensor(out=nr[:], in0=nr[:], in1=V[:],
                                    op=ALU.mult)
            nc.gpsimd.tensor_tensor(out=nr[:], in0=nr[:],
                                    in1=nhalf.broadcast_to([P, 4]),
                                    op=ALU.mult)
            nc.gpsimd.tensor_tensor(out=nr[:], in0=nr[:],
                                    in1=c1p5.broadcast_to([P, 4]),
                                    op=ALU.add)
            Y1 = pool.tile([P, 4], F32, tag="Y1")
            nc.gpsimd.tensor_tensor(out=Y1[:], in0=Y[:], in1=nr[:],
                                    op=ALU.mult)
            # SQpm = [-sqrt(3a), +sqrt(3a(1-e^2))]
            SQ = pool.tile([P, 2], F32, tag="SQ")
            nc.gpsimd.tensor_tensor(out=SQ[:], in0=V[:, 0:2], in1=Y1[:, 0:2],
                                    op=ALU.mult)
            SQpm = pool.tile([P, 2], F32, tag="SQpm")
            nc.gpsimd.tensor_tensor(out=SQpm[:], in0=SQ[:], in1=sgn_pm,
                                    op=ALU.mult)

            # Newton iteration 1
            f1 = pool.tile([P, 1], F32, tag="f1")
            nc.vector.tensor_scalar(out=f1[:], in0=S1[:, 0:1], scalar1=ne[:],
                                    scalar2=F0, op0=ALU.mult, op1=ALU.add)
            d1 = pool.tile([P, 1], F32, tag="d1")
            nc.vector.tensor_scalar(out=d1[:], in0=S1[:, 1:2], scalar1=e_ap,
                                    scalar2=1.0, op0=ALU.mult, op1=ALU.add)
            r1 = pool.tile([P, 1], F32, tag="r1")
            nc.vector.reciprocal(r1[:], d1[:])
            dF1 = pool.tile([P, 1], F32, tag="dF1")
            nc.vector.tensor_tensor(out=dF1[:], in0=f1[:], in1=r1[:],
                                    op=ALU.mult)
            EE1 = pool.tile([P, 2], F32, tag="EE1")
            nc.vector.tensor_tensor(out=EE1[:], in0=EE0[:],
                                    in1=dF1[:].broadcast_to([P, 2]),
                                    op=ALU.subtract)
            S2 = pool.tile([P, 2], F32, tag="S2")
            nc.scalar.activation(S2[:], EE1[:], AF.Sin, bias=m_ap)
            # mm^2 a^3 chain on the idle Scalar engine, emitted AFTER the
            # critical ACTs so the scheduler cannot hoist it above S1/S2
            sa1 = pool.tile([P, 1], F32, tag="sa1")  # mm^2
            nc.scalar.square(sa1[:], mm_ap)
            sa3 = pool.tile([P, 1], F32, tag="sa3")  # mm^2 a
            nc.scalar.mul(sa3[:], sa1[:], a_ap)
            sa2 = pool.tile([P, 1], F32, tag="sa2")  # a^2
            nc.scalar.square(sa2[:], a_ap)
            sa4 = pool.tile([P, 1], F32, tag="sa4")  # mm^2 a^3
            nc.scalar.mul(sa4[:], sa3[:], sa2[:])
            # w2 = -mm^2 a^3 * [x, y]  (sign via -x,-y lanes); the
            # remaining rsqrt(xx+yy)*rci^2 factor lands in the tail.
            w2 = pool.tile([P, 2], F32, tag="w2")
            nc.gpsimd.tensor_tensor(out=w2[:], in0=nxy,
                                    in1=sa4[:].broadcast_to([P, 2]),
                                    op=ALU.mult)

            # GpSimd: rotation-matrix outer products + combine
            z2 = pool.tile([P, 2], F32, tag="z2")  # [-sw*ci, -cw*ci]
            nc.gpsimd.tensor_tensor(out=z2[:], in0=T[:, 5:7],
                                    in1=T[:, 8:9].broadcast_to([P, 2]),
                                    op=ALU.mult)
            C4m = pool.tile([P, 2, 2], F32, tag="C4m")
            nc.gpsimd.tensor_tensor(
                out=C4m[:], in0=nw2[:].unsqueeze(2).broadcast_to([P, 2, 2]),
                in1=LWc[:].unsqueeze(1).broadcast_to([P, 2, 2]), op=ALU.mult)
            Cb4 = pool.tile([P, 2, 2], F32, tag="Cb4")
            nc.gpsimd.tensor_tensor(
                out=Cb4[:], in0=z2[:].unsqueeze(2).broadcast_to([P, 2, 2]),
                in1=T[:, 3:5].unsqueeze(1).broadcast_to([P, 2, 2]),
                op=ALU.mult)
            nc.gpsimd.tensor_tensor(out=Cv, in0=C4m[:], in1=Cb4[:],
                                    op=ALU.add)

            # Newton iteration 2 + first-order trig rotation
            f2 = pool.tile([P, 1], F32, tag="f2")
            nc.vector.tensor_scalar(out=f2[:], in0=S2[:, 0:1], scalar1=ne[:],
                                    scalar2=EE1[:, 0:1], op0=ALU.mult,
                                    op1=ALU.add)
            d2 = pool.tile([P, 1], F32, tag="d2")
            nc.vector.tensor_scalar(out=d2[:], in0=S2[:, 1:2], scalar1=e_ap,
                                    scalar2=1.0, op0=ALU.mult, op1=ALU.add)
            r2 = pool.tile([P, 1], F32, tag="r2")
            nc.vector.reciprocal(r2[:], d2[:])
            dF2 = pool.tile([P, 1], F32, tag="dF2")
            nc.vector.tensor_tensor(out=dF2[:], in0=f2[:], in1=r2[:],
                                    op=ALU.mult)
            # S5 = [sin(Ef), cos(Ef)], Ef = E1 - dF2:
            #   sin ~ S2_0 + dF2*S2_1 ; cos ~ dF2*S2_0 - S2_1
            S5 = pool.tile([P, 2], F32, tag="S5")
            nc.vector.tensor_scalar(out=S5[:, 0:1], in0=S2[:, 1:2],
                                    scalar1=dF2[:], scalar2=S2[:, 0:1],
                                    op0=ALU.mult, op1=ALU.add)
            nc.vector.tensor_scalar(out=S5[:, 1:2], in0=S2[:, 0:1],
                                    scalar1=dF2[:], scalar2=S2[:, 1:2],
                                    op0=ALU.mult, op1=ALU.subtract)

            # ---- tail ----
            rcen = pool.tile([P, 1], F32, tag="rcen")  # a(1 - e cosEf)
            nc.vector.tensor_scalar(out=rcen[:], in0=S5[:, 1:2],
                                    scalar1=nae[:], scalar2=a_ap,
                                    op0=ALU.mult, op1=ALU.add)
            rci = pool.tile([P, 1], F32, tag="rci")
            nc.vector.reciprocal(rci[:], rcen[:])
            PQ = pool.tile([P, 4], F32, tag="PQ")  # [dx, px, dy, qx]
            nc.vector.scalar_tensor_tensor(out=PQ[:, 0:4:2], in0=SQpm[:],
                                           scalar=rci[:], in1=S5[:],
                                           op0=ALU.mult, op1=ALU.mult)
            q = pool.tile([P, 1], F32, tag="q")
            nc.vector.tensor_scalar(out=q[:], in0=Y1[:, 2:3],
                                    scalar1=rci[:], scalar2=rci[:],
                                    op0=ALU.mult, op1=ALU.mult)
            nc.vector.tensor_scalar(out=PQ[:, 1:4:2], in0=w2[:],
                                    scalar1=q[:], scalar2=None,
                                    op0=ALU.mult)

            O1 = pool.tile([P, 6], F32, tag="O1")
            nc.vector.tensor_tensor(
                out=O1[:].rearrange("p (h j) -> p h j", h=2),
                in0=C[:, 0:3].unsqueeze(1).broadcast_to([P, 2, 3]),
                in1=PQ[:, 0:2].unsqueeze(2).broadcast_to([P, 2, 3]),
                op=ALU.mult)
            O2 = pool.tile([P, 6], F32, tag="O2")
            nc.vector.tensor_tensor(
                out=O2[:].rearrange("p (h j) -> p h j", h=2),
                in0=C[:, 3:6].unsqueeze(1).broadcast_to([P, 2, 3]),
                in1=PQ[:, 2:4].unsqueeze(2).broadcast_to([P, 2, 3]),
                op=ALU.mult)
            Ot = pool.tile([P, 6], F32, tag="Ot")
            nc.vector.tensor_tensor(out=Ot[:], in0=O1[:], in1=O2[:],
                                    op=ALU.add)
            nc.sync.dma_start(OUT.ap(), Ot[:])

    nc.compile()
    return nc


def _pack(a, e, i, omega, Omega, mean_motion, mean_anomaly, x):
    P = N_ORBITS
    IN = np.zeros((P, N_IN), np.float32)
    M = np.full((P,), np.float32(mean_anomaly), np.float32)
    w = np.asarray(omega, np.float32).reshape(P)
    W = np.asarray(Omega, np.float32).reshape(P)
    ii = np.asarray(i, np.float32).reshape(P)
    xf = np.asarray(x, np.float32)
    IN[:, 0] = M
    IN[:, 1] = w - HALF_PI
    IN[:, 2] = w
    IN[:, 3] = W
    IN[:, 4] = W - HALF_PI
    IN[:, 5] = w
    IN[:, 6] = HALF_PI - w
    IN[:, 7] = ii
    IN[:, 8] = ii - HALF_PI
    IN[:, 9] = W - HALF_PI
    IN[:, 10] = W
    IN[:, 11] = np.asarray(e, np.float32).reshape(P)
    IN[:, 12] = np.asarray(a, np.float32).reshape(P)
    IN[:, 13] = np.asarray(mean_motion, np.float32).reshape(P)
    IN[:, 14] = xf[:, 0]
    IN[:, 15] = xf[:, 1]
    IN[:, 16] = 0.0
    IN[:, 17] = -HALF_PI
    IN[:, 18] = -1.0
    IN[:, 19] = 1.0
    IN[:, 20] = 3.0
    IN[:, 21] = -0.5
    IN[:, 22] = 1.5
    IN[:, 23] = -xf[:, 0]
    IN[:, 24] = -xf[:, 1]
    return IN


def kernel(a, e, i, omega, Omega, mean_motion, mean_anomaly, x, _trace=False):
    from concourse.bass_utils import run_bass_kernel_spmd

    if "nc" not in _cache:
        _cache["nc"] = _build()
    nc = _cache["nc"]

    IN = _pack(a, e, i, omega, Omega, mean_motion, mean_anomaly, x)
    n_cores = 1 if _trace else 8
    res = run_bass_kernel_spmd(nc, [{"inp": IN}] * n_cores,
                               core_ids=list(range(n_cores)), trace=_trace)
    out = res.results[0]["out"].astype(np.float32)
    if _trace:
        _cache["last_result"] = res
    return out
